# revision 2
# baseline (speedup 1.0000x reference)
"""Trainium2 Bass kernel v3 for nn_DegreeEmbeddingNetwork (gnn_message_passing).

See design notes at bottom.  Strategy: aggregate-first.  Per edge only
h2 = silu(LN(x @ W1c)) is computed (64 wide); the 4 attr-weighted node
aggregates S_m' = sum_e a[e,m'] h2[e] are formed by one PE matmul per tile
against a sparse per-tile weight matrix Dall (one nonzero 4-block per lane,
position fixed by the host's slot packing), and the output projection runs
once per node-slot after aggregation.
"""

import math
import sys

sys.path.insert(0, "/opt/trn_rl_repo")

import numpy as np
import ml_dtypes

import concourse.bacc as bacc
import concourse.tile as tile
from concourse import mybir
from concourse.ap import AP
from concourse.bass_utils import run_bass_kernel_spmd

F32 = mybir.dt.float32
BF16 = mybir.dt.bfloat16
I32 = mybir.dt.int32

N_CORES = 8
MUL0, MUL1 = 64, 32
D_EMB = 160
AVG_AGG = 32.0
LN_EPS = 1e-5
NSLOT = 16          # slots per window
LPS = 128 // NSLOT  # lanes per slot = 8
G = int(__import__("os").environ.get("KV3_G", "24"))  # tiles per group

CONFIG = {
    "dall_engine": "gpsimd",   # "gpsimd" | "vector"
    "sq_dve_tiles": 0,          # tiles per group squared on DVE (rest on ACT)
    "o_copy_engine": "vector",  # PSUM->SBUF copies for O
}

_PROGRAM_CACHE = {}
_LAST_IN_MAPS = None


def build_program(NT, win_of_tile, NW, general_affine):
    nc = bacc.Bacc("TRN2", target_bir_lowering=False, debug=False,
                   num_devices=N_CORES)

    C = NT * 128
    xt_d = nc.dram_tensor("xt", [64, C], BF16, kind="ExternalInput").ap()
    c_d = nc.dram_tensor("cvals", [128, NT * 4], BF16, kind="ExternalInput").ap()
    w1_d = nc.dram_tensor("w1c", [64, 64], BF16, kind="ExternalInput").ap()
    m4_d = nc.dram_tensor("m4", [128, 64], BF16, kind="ExternalInput").ap()
    b0_d = nc.dram_tensor("b0k", [64, 64], BF16, kind="ExternalInput").ap()
    b1_d = nc.dram_tensor("b1k", [64, 32], BF16, kind="ExternalInput").ap()
    if general_affine:
        gb_d = nc.dram_tensor("gbt", [128, 128], BF16, kind="ExternalInput").ap()
    o0_d = nc.dram_tensor("o0", [64, NW * NSLOT], F32, kind="ExternalOutput").ap()
    o1_d = [nc.dram_tensor(f"o1{m}", [32, NW * NSLOT], F32,
                           kind="ExternalOutput").ap() for m in range(3)]

    NG = NT // G
    assert NT % G == 0 and NW % 8 == 0

    first_tile = {}
    last_tile = {}
    for t, w in enumerate(win_of_tile):
        if w not in first_tile:
            first_tile[w] = t
        last_tile[w] = t

    with tile.TileContext(nc) as tc:
        with (
            tc.tile_pool(name="consts", bufs=1) as cpool,
            tc.tile_pool(name="xt", bufs=3) as xt_pool,
            tc.tile_pool(name="cv", bufs=3) as cv_pool,
            tc.tile_pool(name="sq", bufs=2) as sq_pool,
            tc.tile_pool(name="st", bufs=2) as st_pool,
            tc.tile_pool(name="n4", bufs=2) as n4_pool,
            tc.tile_pool(name="h2", bufs=2) as h2_pool,
            tc.tile_pool(name="dall", bufs=2) as dall_pool,
            tc.tile_pool(name="stash", bufs=1) as stash_pool,
            tc.tile_pool(name="osb", bufs=2) as osb_pool,
            tc.tile_pool(name="psH", bufs=int(__import__("os").environ.get("KV3_HB", "2")), space="PSUM") as psH,
            tc.tile_pool(name="psS", bufs=1, space="PSUM") as psS,
            tc.tile_pool(name="psO", bufs=1, space="PSUM") as psO,
        ):
            w1_sb = cpool.tile([64, 64], BF16)
            nc.sync.dma_start(w1_sb[:], w1_d[:])
            m4_sb = cpool.tile([128, 64], BF16)
            nc.sync.dma_start(m4_sb[:], m4_d[:])
            b0_sb = cpool.tile([64, 64], BF16)
            nc.sync.dma_start(b0_sb[:], b0_d[:])
            b1_sb = cpool.tile([64, 32], BF16)
            nc.sync.dma_start(b1_sb[:], b1_d[:])
            if general_affine:
                gb_sb = cpool.tile([128, 128], BF16)
                nc.sync.dma_start(gb_sb[:], gb_d[:])

            stash = stash_pool.tile([64, NW * 64], BF16, tag="stash")

            St = None
            st_bank = None
            bank_lo = 0
            pipeline = []
            state = {"St": None, "st_bank": None, "bank_lo": 0}

            def tail_stage(t0, N4, cvg):
                H2 = h2_pool.tile([128, G * 64], BF16)
                if "silu" not in ABLATE:
                    nc.scalar.activation(H2[:], N4[:],
                                         mybir.ActivationFunctionType.Silu)

                dall = dall_pool.tile([128, G * 64], BF16)
                m4b = AP(tensor=m4_sb[:].tensor, offset=m4_sb[:].offset,
                         ap=[[64, 128], [0, G], [1, 64]])
                cb = AP(tensor=cvg[:].tensor, offset=cvg[:].offset,
                        ap=[[G * 4, 128], [4, G], [0, NSLOT], [1, 4]])
                eng = (nc.gpsimd if CONFIG["dall_engine"] == "gpsimd"
                       else nc.vector)
                if "dall" not in ABLATE:
                    eng.tensor_tensor(
                        dall[:].rearrange("p (t n m) -> p t n m", n=NSLOT, m=4),
                        m4b.rearrange("p t (n m) -> p t n m", m=4),
                        cb, mybir.AluOpType.mult)

                for t in range(G):
                    gt = t0 + t
                    w = win_of_tile[gt]
                    first = first_tile[w] == gt
                    last = last_tile[w] == gt
                    if first:
                        if w % 8 == 0:
                            if state["st_bank"] is not None:
                                nc.scalar.copy(
                                    stash[:, state["bank_lo"] * 64:
                                          (state["bank_lo"] + 8) * 64],
                                    state["st_bank"][:])
                            state["st_bank"] = psS.tile([64, 8 * 64], F32, tag="stbank", name="stbank")
                            state["bank_lo"] = w
                        state["St"] = state["st_bank"][:, (w % 8) * 64:
                                                       (w % 8 + 1) * 64]
                    if "scatter" not in ABLATE:
                        nc.tensor.matmul(
                            state["St"],
                            H2[:, t * 64:(t + 1) * 64],
                            dall[:, t * 64:(t + 1) * 64],
                            start=first, stop=last,
                            skip_group_check=True)

            for g in range(NG):
                t0 = g * G
                xtg = xt_pool.tile([64, G * 128], BF16)
                nc.sync.dma_start(xtg[:], xt_d[:, t0 * 128:(t0 + G) * 128])
                cvg = cv_pool.tile([128, G * 4], BF16)
                nc.sync.dma_start(cvg[:], c_d[:, t0 * 4:(t0 + G) * 4])

                H = psH.tile([128, G * 64], F32)
                for t in range(G):
                    nc.tensor.matmul(
                        H[:, t * 64:(t + 1) * 64],
                        xtg[:, t * 128:(t + 1) * 128],
                        w1_sb[:],
                        start=True, stop=True)

                sq = sq_pool.tile([128, G * 64], BF16)
                nc.scalar.activation(sq[:], H[:],
                                     mybir.ActivationFunctionType.Square)
                ssq = st_pool.tile([128, G], F32, tag="ssq")
                nc.vector.tensor_reduce(
                    ssq[:], sq[:].rearrange("p (t f) -> p t f", f=64),
                    axis=mybir.AxisListType.X, op=mybir.AluOpType.add)
                v = st_pool.tile([128, G], F32, tag="v")
                nc.vector.tensor_scalar(v[:], ssq[:], 1.0, 64.0 * LN_EPS,
                                        mybir.AluOpType.mult,
                                        mybir.AluOpType.add)
                y = st_pool.tile([128, G], F32, tag="y")
                t1 = st_pool.tile([128, G], F32, tag="t1")
                nc.vector.tensor_scalar(t1[:].bitcast(I32), v[:].bitcast(I32),
                                        1, None,
                                        mybir.AluOpType.logical_shift_right)
                nc.vector.tensor_scalar(y[:].bitcast(I32), t1[:].bitcast(I32),
                                        0x5f3759df, -1,
                                        mybir.AluOpType.subtract,
                                        mybir.AluOpType.mult)
                nc.vector.scalar_tensor_tensor(t1[:], y[:], 1.0, y[:],
                                               mybir.AluOpType.mult,
                                               mybir.AluOpType.mult)
                nc.vector.scalar_tensor_tensor(t1[:], t1[:], -0.5, v[:],
                                               mybir.AluOpType.mult,
                                               mybir.AluOpType.mult)
                nc.vector.scalar_tensor_tensor(y[:], t1[:], 1.5, y[:],
                                               mybir.AluOpType.add,
                                               mybir.AluOpType.mult)
                nc.vector.scalar_tensor_tensor(t1[:], y[:], 1.0, y[:],
                                               mybir.AluOpType.mult,
                                               mybir.AluOpType.mult)
                nc.vector.scalar_tensor_tensor(t1[:], t1[:], -4.0, v[:],
                                               mybir.AluOpType.mult,
                                               mybir.AluOpType.mult)
                nc.vector.scalar_tensor_tensor(y[:], t1[:], 12.0, y[:],
                                               mybir.AluOpType.add,
                                               mybir.AluOpType.mult)

                N4 = n4_pool.tile([128, G * 64], BF16)
                nc.vector.tensor_tensor(
                    N4[:].rearrange("p (t f) -> p t f", f=64),
                    H[:].rearrange("p (t f) -> p t f", f=64),
                    y[:].unsqueeze(2).broadcast_to([128, G, 64]),
                    mybir.AluOpType.mult)
                if general_affine:
                    gex = AP(tensor=gb_sb[:].tensor, offset=gb_sb[:].offset,
                             ap=[[128, 128], [0, G], [1, 64]])
                    bex = AP(tensor=gb_sb[:].tensor, offset=gb_sb[:].offset + 64,
                             ap=[[128, 128], [0, G], [1, 64]])
                    nc.vector.tensor_tensor(
                        N4[:].rearrange("p (t f) -> p t f", f=64),
                        N4[:].rearrange("p (t f) -> p t f", f=64),
                        gex.rearrange("p t f -> p t f"),
                        mybir.AluOpType.mult)
                    nc.vector.tensor_tensor(
                        N4[:].rearrange("p (t f) -> p t f", f=64),
                        N4[:].rearrange("p (t f) -> p t f", f=64),
                        bex.rearrange("p t f -> p t f"),
                        mybir.AluOpType.add)

                prev = (t0, N4, cvg)
                if g > 0:
                    tail_stage(*pipeline[0])
                    pipeline[0] = prev
                else:
                    pipeline.append(prev)

            tail_stage(*pipeline[0])
            nc.scalar.copy(stash[:, state["bank_lo"] * 64:
                                 (state["bank_lo"] + 8) * 64],
                           state["st_bank"][:])

            CW = 32
            for lo in range(0, NW, CW):
                hi = min(NW, lo + CW)
                ncols = (hi - lo) * NSLOT
                rhs0 = AP(tensor=stash[:].tensor,
                          offset=stash[:].offset + lo * 64,
                          ap=[[NW * 64, 64], [64, hi - lo], [4, NSLOT]])
                Op = psO.tile([64, 512], F32, tag="op")
                nc.tensor.matmul(Op[:, 0:ncols], b0_sb[:], rhs0,
                                 start=True, stop=True)
                ob = osb_pool.tile([64, 512], F32, tag="osb")
                nc.vector.tensor_copy(ob[:, 0:ncols], Op[:, 0:ncols])
                nc.sync.dma_start(
                    o0_d[:, lo * NSLOT:lo * NSLOT + ncols], ob[:, 0:ncols])
                for m in range(3):
                    rhs1 = AP(tensor=stash[:].tensor,
                              offset=stash[:].offset + lo * 64 + 1 + m,
                              ap=[[NW * 64, 64], [64, hi - lo], [4, NSLOT]])
                    Op1 = psO.tile([64, 512], F32, tag="op")
                    nc.tensor.matmul(Op1[0:32, 0:ncols], b1_sb[:], rhs1,
                                     start=True, stop=True)
                    ob1 = osb_pool.tile([64, 512], F32, tag="osb")
                    nc.scalar.copy(ob1[0:32, 0:ncols], Op1[0:32, 0:ncols])
                    nc.sync.dma_start(
                        o1_d[m][:, lo * NSLOT:lo * NSLOT + ncols],
                        ob1[0:32, 0:ncols])

    nc.finalize()
    return nc


def _pack_core(order_by_node, lo, hi):
    """Split each node's edges into 8-multiple chunks + one tail chunk,
    sort desc, group NSLOT per window.  Returns list of (T_w, grp) where
    grp = list of (size, node, edge_ids)."""
    chunks = []
    for n in range(lo, hi):
        e = order_by_node[n]
        c = len(e)
        if c == 0:
            continue
        p = 0
        full = (c // LPS) * LPS
        while p < full:
            step = min(64, full - p)
            chunks.append((step, n, e[p:p + step]))
            p += step
        if c - p > 0:
            chunks.append((c - p, n, e[p:]))
    chunks.sort(key=lambda x: -x[0])
    windows = []
    for i in range(0, len(chunks), NSLOT):
        grp = chunks[i:i + NSLOT]
        T_w = max(1, (grp[0][0] + LPS - 1) // LPS)
        windows.append((T_w, grp))
    return windows


def kernel(dst_input, src_attr, scalars, lin_w, lin_b, rad_w1, rad_g, rad_beta,
           rad_w2, rad_off, proj_w0, proj_b0, proj_w1, dst_index):
    dst_input = np.asarray(dst_input)
    src_attr = np.asarray(src_attr, np.float32)
    scalars = np.asarray(scalars, np.float32)
    lin_w = np.asarray(lin_w, np.float64)
    lin_b = np.asarray(lin_b, np.float64)
    rad_w1 = np.asarray(rad_w1, np.float64)
    rad_g = np.asarray(rad_g, np.float64)
    rad_beta = np.asarray(rad_beta, np.float64)
    rad_w2 = np.asarray(rad_w2, np.float64)
    rad_off = np.asarray(rad_off, np.float64)
    proj_w0 = np.asarray(proj_w0, np.float64)
    proj_b0 = np.asarray(proj_b0, np.float64)
    proj_w1 = np.asarray(proj_w1, np.float64)
    dst_index = np.asarray(dst_index)

    N = dst_input.shape[0]
    out_dtype = dst_input.dtype

    general_affine = not (np.allclose(rad_g, 1.0)
                          and np.allclose(rad_beta, 0.0))

    s0 = lin_w + lin_b
    k0 = 1.0 / (math.sqrt(MUL0 + MUL1) * math.sqrt(AVG_AGG))
    k1 = 1.0 / (math.sqrt(MUL0 + 2 * MUL1) * math.sqrt(AVG_AGG))
    A0 = s0[:, None] * proj_w0[:MUL0, :]
    A1 = s0[:, None] * proj_w1[:MUL0, :]
    B0k = rad_w2[:, 0:64] @ A0 * k0
    B1k = rad_w2[:, 64:128] @ A1 * k1
    c0 = rad_off[0:64] @ A0 * k0
    c1 = rad_off[64:128] @ A1 * k1
    W1c = rad_w1 - rad_w1.mean(axis=1, keepdims=True)

    NPC = (N + N_CORES - 1) // N_CORES
    order = np.argsort(dst_index, kind="stable")
    dst_sorted = dst_index[order]
    starts = np.searchsorted(dst_sorted, np.arange(N + 1))
    order_by_node = [order[starts[n]:starts[n + 1]] for n in range(N)]

    core_windows = [
        _pack_core(order_by_node, k * NPC, min(N, (k + 1) * NPC))
        for k in range(N_CORES)
    ]

    max_nw = max(len(w) for w in core_windows)
    NW = ((max_nw + 7) // 8) * 8
    for w in core_windows:
        while len(w) < NW:
            w.append((1, []))
    Tw_common = [max(core_windows[k][i][0] for k in range(N_CORES))
                 for i in range(NW)]
    NT = sum(Tw_common)
    NT_pad = ((NT + G - 1) // G) * G
    Tw_common[-1] += NT_pad - NT
    NT = NT_pad

    win_of_tile = []
    for i, T in enumerate(Tw_common):
        win_of_tile += [i] * T

    key = (NT, tuple(win_of_tile), NW, general_affine)
    if key not in _PROGRAM_CACHE:
        _PROGRAM_CACHE.clear()
        _PROGRAM_CACHE[key] = build_program(NT, win_of_tile, NW,
                                            general_affine)
    nc = _PROGRAM_CACHE[key]

    m4 = np.zeros((128, 64), np.float32)
    for p in range(128):
        m4[p, (p % NSLOT) * 4:(p % NSLOT) * 4 + 4] = 1.0
    tile_start = np.cumsum([0] + Tw_common)

    in_maps = []
    placements = []
    for k in range(N_CORES):
        xt = np.zeros((NT * 128, 64), np.float32)
        cv = np.zeros((128, NT, 4), np.float32)
        plc = []
        for wi, (T_w, grp) in enumerate(core_windows[k]):
            t0 = tile_start[wi]
            Tw = Tw_common[wi]
            for si, (sz, n, eids) in enumerate(grp):
                if sz == 0:
                    continue
                plc.append((n, wi, si))
                for idx, e in enumerate(eids):
                    j, t = divmod(idx, Tw)
                    p = si + NSLOT * j
                    gt = t0 + t
                    xt[gt * 128 + p] = scalars[e]
                    cv[p, gt, :] = src_attr[e, 0:4]
        m = {
            "xt": np.ascontiguousarray(xt.T).astype(ml_dtypes.bfloat16),
            "cvals": np.ascontiguousarray(
                cv.reshape(128, NT * 4)).astype(ml_dtypes.bfloat16),
            "w1c": W1c.astype(ml_dtypes.bfloat16),
            "m4": m4.astype(ml_dtypes.bfloat16),
            "b0k": B0k.astype(ml_dtypes.bfloat16),
            "b1k": B1k.astype(ml_dtypes.bfloat16),
        }
        if general_affine:
            gbt = np.zeros((128, 128), np.float32)
            gbt[:, 0:64] = rad_g[None, :]
            gbt[:, 64:128] = rad_beta[None, :]
            m["gbt"] = gbt.astype(ml_dtypes.bfloat16)
        in_maps.append(m)
        placements.append(plc)

    global _LAST_IN_MAPS
    _LAST_IN_MAPS = in_maps
    res = run_bass_kernel_spmd(nc, in_maps, core_ids=list(range(N_CORES)))

    out = np.zeros((N, D_EMB), np.float64)
    vidx = 64 + 3 * np.arange(32)
    for k in range(N_CORES):
        o0 = res.results[k]["o0"]
        o1 = [res.results[k][f"o1{m}"] for m in range(3)]
        for (n, wi, si) in placements[k]:
            col = wi * NSLOT + si
            out[n, 0:64] += o0[:, col]
            for m in range(3):
                out[n, vidx + m] += o1[m][:, col]

    if np.any(proj_b0 != 0) or np.any(c0 != 0) or np.any(c1 != 0):
        cnt = np.bincount(dst_index, minlength=N).astype(np.float64)
        suma0 = np.bincount(dst_index,
                            weights=src_attr[:, 0].astype(np.float64),
                            minlength=N)
        out[:, 0:64] += cnt[:, None] * (proj_b0 / math.sqrt(AVG_AGG))[None, :]
        out[:, 0:64] += suma0[:, None] * c0[None, :]
        for m_ in range(3):
            sa = np.bincount(dst_index,
                             weights=src_attr[:, 1 + m_].astype(np.float64),
                             minlength=N)
            out[:, 64 + m_::3][:, 0:32] += sa[:, None] * c1[None, :]

    return out.astype(out_dtype)
